# revision 4
# baseline (speedup 1.0000x reference)
import sys

if "/opt/trn_rl_repo" not in sys.path:
    sys.path.insert(0, "/opt/trn_rl_repo")

import numpy as np

N = 3_000_000
NCORES = 8
NPC = N // NCORES          # 375_000 samples per core
PART = 128                 # SBUF partitions
SPP = 2944                 # samples per partition (padded)
NPADPC = PART * SPP        # 376_832
NT = 2                     # tiles per core
K = SPP // NT              # 1472 samples per tile per partition

# All compute in fp16: DVE is 2x-pumped for 16-bit dtypes (measured 533 ns
# vs 916 ns per TT @736) and the data is well-conditioned (det C in
# [0.42, 2.2], all intermediates < 100), so fp16's 4.9e-4 rounding stays
# far inside tolerance.

# Column split between engines. tensor_tensor never grabs the shared
# DVE/GpSimd SBUF port pair (only 2-port perf-mode ops do), so the two
# engines can run disjoint column ranges concurrently. Balance point from
# the cost model: DVE TT = (KD/2+144) cyc @0.96GHz, Pool TT = KG/0.42 cyc
# @1.2GHz + 156 ns fixed.
KG = 304                   # columns handled by GpSimd (0 = disabled)
KD = K - KG                # columns handled by DVE

# Per-partition DRAM layout: [NT][9 planes][K], so each tile is one
# contiguous [128, 9K] DMA and every SBUF compute view is stride-1.

SQRT02 = 0.4472135954999579  # sqrt(0.2)
SQRT8 = 2.8284271247461903   # sqrt(8)

_cache = {}


def _emit_slice_stages(nc, eng, st, f, pwr, AL, AF):
    """Generator emitting the per-sample gradient schedule on one engine.

    Yields at stage boundaries so two slices (DVE + GpSimd) can be emitted
    interleaved; with per-engine in-order queues, round-robin emission keeps
    every queue in rough consumption order (whole-slice emission serializes
    the engines through the shared ACT ops and the GpSimd slice's DVE
    reciprocal).

    eng: BassEitherVectorEngine (nc.vector or nc.gpsimd)
    st(tag, bufs): scratch tile factory for this slice width
    f: list of 9 input component views
    pwr(idx): output component view for final writes
    """
    # No scalar_tensor_tensor / tensor_scalar anywhere: TensorScalarPtr fails
    # the V3 ISA check on the Pool engine, so all scalar constants are folded
    # into ACT-engine ops (Copy scale/bias, Square scale) instead.
    TT = eng.tensor_tensor
    ACT = nc.scalar.activation

    # s0: squares of the 9 F entries (ACT engine)
    sf = []
    for i in range(9):
        s = st(f"sf{i}", 1)
        ACT(s, f[i], AF.Square)
        sf.append(s)
    yield

    # s1: C off-diag first - it reads raw F so the vector engine can start
    # before the ACT sf squares land.  C = F^T F.
    c = {}
    for (name, prs) in (("c01", ((0, 1), (3, 4), (6, 7))),
                        ("c02", ((0, 2), (3, 5), (6, 8))),
                        ("c12", ((1, 2), (4, 5), (7, 8)))):
        m1 = st("tmpA", 2)
        TT(m1, f[prs[0][0]], f[prs[0][1]], AL.mult)
        m2 = st("tmpB", 2)
        TT(m2, f[prs[1][0]], f[prs[1][1]], AL.mult)
        s12 = st("tmpC", 2)
        TT(s12, m1, m2, AL.add)
        m3 = st("tmpA", 2)
        TT(m3, f[prs[2][0]], f[prs[2][1]], AL.mult)
        co = st(name, 1)
        TT(co, s12, m3, AL.add)
        c[name] = co
    yield

    # s2: C diag + t2 = 8 c00 + c11 + c22 = 2 I4 ; gamma diag terms
    for (name, i0, i1, i2) in (("c00", 0, 3, 6), ("c11", 1, 4, 7), ("c22", 2, 5, 8)):
        q = st("tmpA", 2)
        TT(q, sf[i0], sf[i1], AL.add)
        cd = st(name, 1)
        TT(cd, q, sf[i2], AL.add)
        c[name] = cd
    e8 = st("tmpA", 2)
    ACT(e8, c["c00"], AF.Copy, scale=8.0)
    q = st("tmpC", 2)
    TT(q, e8, c["c11"], AL.add)
    t2b = st("tmpB", 2)
    TT(t2b, q, c["c22"], AL.add)
    g0 = st("g0", 1)
    ACT(g0, t2b, AF.Copy, bias=16.0, scale=1.6)
    g12 = st("g12", 1)
    ACT(g12, t2b, AF.Copy, bias=16.0, scale=0.2)
    yield

    # s3: squares of C off-diag (ACT) + A = cof(C) diag
    sqc = {}
    for i, name in enumerate(("c01", "c02", "c12")):
        s = st(f"sf{i}", 1)
        ACT(s, c[name], AF.Square)
        sqc[name] = s
    a = {}
    for (name, x0, x1, sq) in (("a00", "c11", "c22", "c12"),
                               ("a11", "c00", "c22", "c02"),
                               ("a22", "c00", "c11", "c01")):
        m = st("tmpA", 2)
        TT(m, c[x0], c[x1], AL.mult)
        ad = st(name, 1)
        TT(ad, m, sqc[sq], AL.subtract)
        a[name] = ad
    yield

    # s4: A off-diag
    for (name, p0, p1) in (("a01", ("c02", "c12"), ("c01", "c22")),
                           ("a02", ("c01", "c12"), ("c02", "c11")),
                           ("a12", ("c01", "c02"), ("c00", "c12"))):
        m1 = st("tmpA", 2)
        TT(m1, c[p0[0]], c[p0[1]], AL.mult)
        m2 = st("tmpB", 2)
        TT(m2, c[p1[0]], c[p1[1]], AL.mult)
        ao = st(name, 1)
        TT(ao, m1, m2, AL.subtract)
        a[name] = ao
    yield

    # s5: I3 = det C ; r3 = 1/I3 (reciprocal always on DVE; for the GpSimd
    # slice this single small op lands mid-queue on DVE right after the DVE
    # slice's own recip, so neither engine waits long)
    m1 = st("tmpA", 2)
    TT(m1, c["c00"], a["a00"], AL.mult)
    m2 = st("tmpB", 2)
    TT(m2, c["c01"], a["a01"], AL.mult)
    s12 = st("tmpC", 2)
    TT(s12, m1, m2, AL.add)
    m3 = st("tmpA", 2)
    TT(m3, c["c02"], a["a02"], AL.mult)
    i3 = st("tmpB", 2)
    TT(i3, s12, m3, AL.add)
    # reciprocal_approx_fast is fp32-only; bounce through fp32 on ACT
    i3f = st("i3f", 1, wide=True)
    ACT(i3f, i3, AF.Copy)
    r3f = st("r3f", 1, wide=True)
    nc.vector.reciprocal_approx_fast(r3f, i3f)
    r3 = st("r3", 1)
    ACT(r3, r3f, AF.Copy)
    yield

    # s6: t3 = 8 a00 + a11 + a22 = 2 I5 ; kappa/lambda scalars; sqa squares
    e8a00 = st("e8a00", 1)
    ACT(e8a00, a["a00"], AF.Copy, scale=8.0)
    q = st("tmpA", 2)
    TT(q, e8a00, a["a11"], AL.add)
    t3 = st("t3", 1)
    TT(t3, q, a["a22"], AL.add)
    # xk20 = kappa = (0.2 t3^2 - 56) r3 + 20
    sq3 = st("tmpA", 2)
    ACT(sq3, t3, AF.Square, scale=SQRT02)
    sq3m = st("tmpB", 2)
    ACT(sq3m, sq3, AF.Copy, bias=-56.0)
    xkr = st("tmpC", 2)
    TT(xkr, sq3m, r3, AL.mult)
    xk20 = st("xk", 1)
    ACT(xk20, xkr, AF.Copy, bias=20.0)
    # lamm = -0.2 t3 r3 = lambda coefficient on That
    t3m = st("tmpA", 2)
    ACT(t3m, t3, AF.Copy, scale=-0.2)
    lamm = st("lam", 1)
    TT(lamm, t3m, r3, AL.mult)
    # e8a01 for That off-diag th12
    e8a01 = st("e8a01", 1)
    ACT(e8a01, a["a01"], AF.Copy, scale=8.0)
    # squares of A entries (ACT), into sf3..sf8 slots
    sqa = {}
    for i, name in enumerate(("a00", "a01", "a02", "a11", "a12", "a22")):
        s = st(f"sf{i + 3}", 1)
        ACT(s, a[name], AF.Square)
        sqa[name] = s
    yield

    # s7: That = 2*AGA diag (into c00/c11/c22 slots)
    th = {}
    for (tag, s0, s1, s2) in (("c00", "a00", "a01", "a02"),
                              ("c11", "a01", "a11", "a12"),
                              ("c22", "a02", "a12", "a22")):
        q8 = st("tmpA", 2)
        ACT(q8, a[s0], AF.Square, scale=SQRT8)
        q = st("tmpB", 2)
        TT(q, q8, sqa[s1], AL.add)
        tt = st(tag, 1)
        TT(tt, q, sqa[s2], AL.add)
        th[tag] = tt
    yield

    # s8: That off-diag (into c01/c02/c12 slots)
    for (tag, e8t, pm, p1, p2) in (
            ("c01", e8a00, "a01", ("a01", "a11"), ("a02", "a12")),
            ("c02", e8a00, "a02", ("a01", "a12"), ("a02", "a22")),
            ("c12", e8a01, "a02", ("a11", "a12"), ("a12", "a22"))):
        m1 = st("tmpA", 2)
        TT(m1, e8t, a[pm], AL.mult)
        m2 = st("tmpB", 2)
        TT(m2, a[p1[0]], a[p1[1]], AL.mult)
        s12 = st("tmpC", 2)
        TT(s12, m1, m2, AL.add)
        m3 = st("tmpA", 2)
        TT(m3, a[p2[0]], a[p2[1]], AL.mult)
        tt = st(tag, 1)
        TT(tt, s12, m3, AL.add)
        th[tag] = tt
    yield

    # s9: S diag (into sf3..sf5 slots)
    sS = {}
    for (sname, tago, aname, thtag, g) in (
            ("s00", "sf3", "a00", "c00", g0),
            ("s11", "sf4", "a11", "c11", g12),
            ("s22", "sf5", "a22", "c22", g12)):
        k1 = st("tmpA", 2)
        TT(k1, xk20, a[aname], AL.mult)
        k2 = st("tmpB", 2)
        TT(k2, lamm, th[thtag], AL.mult)
        ks = st("tmpC", 2)
        TT(ks, k1, k2, AL.add)
        so = st(tago, 1)
        TT(so, ks, g, AL.add)
        sS[sname] = so
    yield

    # s10: S off-diag (into sf6..sf8 slots)
    for (sname, tago, aname, thtag) in (
            ("s01", "sf6", "a01", "c01"),
            ("s02", "sf7", "a02", "c02"),
            ("s12", "sf8", "a12", "c12")):
        k1 = st("tmpA", 2)
        TT(k1, xk20, a[aname], AL.mult)
        k2 = st("tmpB", 2)
        TT(k2, lamm, th[thtag], AL.mult)
        so = st(tago, 1)
        TT(so, k1, k2, AL.add)
        sS[sname] = so
    yield

    # s11..s13: P = F S (S symmetric), one output row per stage
    Smat = [[sS["s00"], sS["s01"], sS["s02"]],
            [sS["s01"], sS["s11"], sS["s12"]],
            [sS["s02"], sS["s12"], sS["s22"]]]
    for r in range(3):
        for j in range(3):
            m1 = st("tmpA", 2)
            TT(m1, f[3 * r + 0], Smat[0][j], AL.mult)
            m2 = st("tmpB", 2)
            TT(m2, f[3 * r + 1], Smat[1][j], AL.mult)
            s12 = st("tmpC", 2)
            TT(s12, m1, m2, AL.add)
            m3 = st("tmpA", 2)
            TT(m3, f[3 * r + 2], Smat[2][j], AL.mult)
            TT(pwr(3 * r + j), s12, m3, AL.add)
        yield


def _build():
    import concourse.bass as bass
    import concourse.tile as tile
    from concourse import bacc, mybir
    from contextlib import ExitStack

    f16 = mybir.dt.float16
    AL = mybir.AluOpType
    AF = mybir.ActivationFunctionType

    ROW = NT * 9 * K
    nc = bacc.Bacc("TRN2", target_bir_lowering=False, debug=False)
    fin_d = nc.dram_tensor("fin", [PART, ROW], f16, kind="ExternalInput").ap()
    pout_d = nc.dram_tensor("pout", [PART, ROW], f16, kind="ExternalOutput").ap()

    with tile.TileContext(nc) as tc:
        with ExitStack() as ctx:
            io = ctx.enter_context(tc.tile_pool(name="io", bufs=2))
            sp = ctx.enter_context(tc.tile_pool(name="sp", bufs=1))

            # Issue all input DMAs up front: the tile-t+1 load must not queue
            # behind the tile-t store's semaphore wait on the SP sequencer.
            fcs = []
            for t in range(NT):
                ft = io.tile([PART, 9 * K], f16, name="fin", tag="fin", bufs=2)
                nc.sync.dma_start(ft, fin_d[:, t * 9 * K:(t + 1) * 9 * K])
                fcs.append(ft)

            for t in range(NT):
                fc = fcs[t]
                pc = io.tile([PART, 9 * K], f16, name="pout", tag="pout",
                             bufs=1)

                slices = [("A", nc.vector, 0, KD)]
                if KG:
                    slices.append(("B", nc.gpsimd, KD, KG))
                gens = []
                for (sfx, eng, lo, w) in slices:
                    fv = [fc[:, i * K + lo: i * K + lo + w] for i in range(9)]

                    def st(tag, bufs, wide=False, _sfx=sfx, _w=w):
                        nm = f"{tag}{_sfx}"
                        dt = mybir.dt.float32 if wide else f16
                        return sp.tile([PART, _w], dt, name=nm, tag=nm, bufs=bufs)

                    def pwr(idx, _lo=lo, _w=w):
                        return pc[:, idx * K + _lo: idx * K + _lo + _w]

                    gens.append(_emit_slice_stages(nc, eng, st, fv, pwr, AL, AF))

                # Round-robin the stage emission across the engine slices.
                alive = list(gens)
                while alive:
                    nxt = []
                    for g in alive:
                        try:
                            next(g)
                            nxt.append(g)
                        except StopIteration:
                            pass
                    alive = nxt
                for r in range(3):
                    nc.sync.dma_start(
                        pout_d[:, t * 9 * K + 3 * r * K: t * 9 * K + (3 * r + 3) * K],
                        pc[:, 3 * r * K:(3 * r + 3) * K])

    nc.compile()
    return nc


def _get_nc():
    if "nc" not in _cache:
        _cache["nc"] = _build()
    return _cache["nc"]


def _make_in_maps(F):
    x = F.reshape(N, 9).astype(np.float16)
    eye9 = np.array([1, 0, 0, 0, 1, 0, 0, 0, 1], dtype=np.float16)
    pad = np.tile(eye9, (NPADPC - NPC, 1))
    in_maps = []
    for cidx in range(NCORES):
        xc = x[cidx * NPC:(cidx + 1) * NPC]
        xcp = (np.concatenate([xc, pad], axis=0)
               .reshape(PART, NT, K, 9).transpose(0, 1, 3, 2)
               .reshape(PART, NT * 9 * K))
        in_maps.append({"fin": np.ascontiguousarray(xcp)})
    return in_maps


def kernel(**inputs):
    from concourse.bass_utils import run_bass_kernel_spmd

    F = np.asarray(inputs["F"], dtype=np.float32)
    nc = _get_nc()
    in_maps = _make_in_maps(F)

    res = run_bass_kernel_spmd(nc, in_maps, list(range(NCORES)))

    out = np.empty((N, 9), dtype=np.float32)
    for cidx in range(NCORES):
        oc = (np.asarray(res.results[cidx]["pout"]).astype(np.float32)
              .reshape(PART, NT, 9, K).transpose(0, 1, 3, 2)
              .reshape(NPADPC, 9))
        out[cidx * NPC:(cidx + 1) * NPC] = oc[:NPC]
    return out.reshape(N, 3, 3)


# revision 5
# speedup vs baseline: 1.4582x; 1.4582x over previous
import sys

if "/opt/trn_rl_repo" not in sys.path:
    sys.path.insert(0, "/opt/trn_rl_repo")

import numpy as np

N = 3_000_000
NCORES = 8
NPC = N // NCORES          # 375_000 samples per core
PART = 128                 # SBUF partitions
SPP = 2944                 # samples per partition (padded)
NPADPC = PART * SPP        # 376_832
NT = 2                     # tiles per core
K = SPP // NT              # 1472 samples per tile per partition

# All compute in fp16: DVE is 2x-pumped for 16-bit dtypes (measured 533 ns
# vs 916 ns per TT @736) and the data is well-conditioned (det C in
# [0.42, 2.2], all intermediates < 100), so fp16's 4.9e-4 rounding stays
# far inside tolerance.

# Column split between engines. tensor_tensor never grabs the shared
# DVE/GpSimd SBUF port pair (only 2-port perf-mode ops do), so the two
# engines can run disjoint column ranges concurrently. Balance point from
# the cost model: DVE TT = (KD/2+144) cyc @0.96GHz, Pool TT = KG/0.42 cyc
# @1.2GHz + 156 ns fixed.
# Measured on HW: co-running GpSimd TT with DVE TT inflates BOTH engines'
# per-op time ~+420 ns (DVE TT reads its second operand through the shared
# DVE/GpSimd SBUF port pair, which is an exclusive per-instruction lock), so
# the offload is a net loss: KG=304 measured 386 us vs 260 us DVE-only.
KG = 0                     # columns handled by GpSimd (0 = disabled)
KD = K - KG                # columns handled by DVE

# Per-partition DRAM layout: [NT][9 planes][K], so each tile is one
# contiguous [128, 9K] DMA and every SBUF compute view is stride-1.

SQRT02 = 0.4472135954999579  # sqrt(0.2)
SQRT8 = 2.8284271247461903   # sqrt(8)

_cache = {}


def _emit_slice_stages(nc, eng, st, f, pwr, AL, AF):
    """Generator emitting the per-sample gradient schedule on one engine.

    Yields at stage boundaries so two slices (DVE + GpSimd) can be emitted
    interleaved; with per-engine in-order queues, round-robin emission keeps
    every queue in rough consumption order (whole-slice emission serializes
    the engines through the shared ACT ops and the GpSimd slice's DVE
    reciprocal).

    eng: BassEitherVectorEngine (nc.vector or nc.gpsimd)
    st(tag, bufs): scratch tile factory for this slice width
    f: list of 9 input component views
    pwr(idx): output component view for final writes
    """
    # No scalar_tensor_tensor / tensor_scalar anywhere: TensorScalarPtr fails
    # the V3 ISA check on the Pool engine, so all scalar constants are folded
    # into ACT-engine ops (Copy scale/bias, Square scale) instead.
    TT = eng.tensor_tensor
    ACT = nc.scalar.activation

    # s0: squares of the 9 F entries (ACT engine)
    sf = []
    for i in range(9):
        s = st(f"sf{i}", 1)
        ACT(s, f[i], AF.Square)
        sf.append(s)
    yield

    # s1: C off-diag first - it reads raw F so the vector engine can start
    # before the ACT sf squares land.  C = F^T F.
    c = {}
    for (name, prs) in (("c01", ((0, 1), (3, 4), (6, 7))),
                        ("c02", ((0, 2), (3, 5), (6, 8))),
                        ("c12", ((1, 2), (4, 5), (7, 8)))):
        m1 = st("tmpA", 2)
        TT(m1, f[prs[0][0]], f[prs[0][1]], AL.mult)
        m2 = st("tmpB", 2)
        TT(m2, f[prs[1][0]], f[prs[1][1]], AL.mult)
        s12 = st("tmpC", 2)
        TT(s12, m1, m2, AL.add)
        m3 = st("tmpA", 2)
        TT(m3, f[prs[2][0]], f[prs[2][1]], AL.mult)
        co = st(name, 1)
        TT(co, s12, m3, AL.add)
        c[name] = co
    yield

    # s2: C diag + t2 = 8 c00 + c11 + c22 = 2 I4 ; gamma diag terms
    for (name, i0, i1, i2) in (("c00", 0, 3, 6), ("c11", 1, 4, 7), ("c22", 2, 5, 8)):
        q = st("tmpA", 2)
        TT(q, sf[i0], sf[i1], AL.add)
        cd = st(name, 1)
        TT(cd, q, sf[i2], AL.add)
        c[name] = cd
    e8 = st("tmpA", 2)
    ACT(e8, c["c00"], AF.Copy, scale=8.0)
    q = st("tmpC", 2)
    TT(q, e8, c["c11"], AL.add)
    t2b = st("tmpB", 2)
    TT(t2b, q, c["c22"], AL.add)
    g0 = st("g0", 1)
    ACT(g0, t2b, AF.Copy, bias=16.0, scale=1.6)
    g12 = st("g12", 1)
    ACT(g12, t2b, AF.Copy, bias=16.0, scale=0.2)
    yield

    # s3: squares of C off-diag (ACT) + A = cof(C) diag
    sqc = {}
    for i, name in enumerate(("c01", "c02", "c12")):
        s = st(f"sf{i}", 1)
        ACT(s, c[name], AF.Square)
        sqc[name] = s
    a = {}
    for (name, x0, x1, sq) in (("a00", "c11", "c22", "c12"),
                               ("a11", "c00", "c22", "c02"),
                               ("a22", "c00", "c11", "c01")):
        m = st("tmpA", 2)
        TT(m, c[x0], c[x1], AL.mult)
        ad = st(name, 1)
        TT(ad, m, sqc[sq], AL.subtract)
        a[name] = ad
    yield

    # s4: A off-diag
    for (name, p0, p1) in (("a01", ("c02", "c12"), ("c01", "c22")),
                           ("a02", ("c01", "c12"), ("c02", "c11")),
                           ("a12", ("c01", "c02"), ("c00", "c12"))):
        m1 = st("tmpA", 2)
        TT(m1, c[p0[0]], c[p0[1]], AL.mult)
        m2 = st("tmpB", 2)
        TT(m2, c[p1[0]], c[p1[1]], AL.mult)
        ao = st(name, 1)
        TT(ao, m1, m2, AL.subtract)
        a[name] = ao
    yield

    # s5: I3 = det C ; r3 = 1/I3 (reciprocal always on DVE; for the GpSimd
    # slice this single small op lands mid-queue on DVE right after the DVE
    # slice's own recip, so neither engine waits long)
    m1 = st("tmpA", 2)
    TT(m1, c["c00"], a["a00"], AL.mult)
    m2 = st("tmpB", 2)
    TT(m2, c["c01"], a["a01"], AL.mult)
    s12 = st("tmpC", 2)
    TT(s12, m1, m2, AL.add)
    m3 = st("tmpA", 2)
    TT(m3, c["c02"], a["a02"], AL.mult)
    i3 = st("tmpB", 2)
    TT(i3, s12, m3, AL.add)
    # reciprocal_approx_fast is fp32-only; bounce through fp32 on ACT
    i3f = st("i3f", 1, wide=True)
    ACT(i3f, i3, AF.Copy)
    r3f = st("r3f", 1, wide=True)
    nc.vector.reciprocal_approx_fast(r3f, i3f)
    r3 = st("r3", 1)
    ACT(r3, r3f, AF.Copy)
    yield

    # s6: t3 = 8 a00 + a11 + a22 = 2 I5 ; kappa/lambda scalars; sqa squares
    e8a00 = st("e8a00", 1)
    ACT(e8a00, a["a00"], AF.Copy, scale=8.0)
    q = st("tmpA", 2)
    TT(q, e8a00, a["a11"], AL.add)
    t3 = st("t3", 1)
    TT(t3, q, a["a22"], AL.add)
    # xk20 = kappa = (0.2 t3^2 - 56) r3 + 20
    sq3 = st("tmpA", 2)
    ACT(sq3, t3, AF.Square, scale=SQRT02)
    sq3m = st("tmpB", 2)
    ACT(sq3m, sq3, AF.Copy, bias=-56.0)
    xkr = st("tmpC", 2)
    TT(xkr, sq3m, r3, AL.mult)
    xk20 = st("xk", 1)
    ACT(xk20, xkr, AF.Copy, bias=20.0)
    # lamm = -0.2 t3 r3 = lambda coefficient on That
    t3m = st("tmpA", 2)
    ACT(t3m, t3, AF.Copy, scale=-0.2)
    lamm = st("lam", 1)
    TT(lamm, t3m, r3, AL.mult)
    # e8a01 for That off-diag th12
    e8a01 = st("e8a01", 1)
    ACT(e8a01, a["a01"], AF.Copy, scale=8.0)
    # squares of A entries (ACT), into sf3..sf8 slots
    sqa = {}
    for i, name in enumerate(("a00", "a01", "a02", "a11", "a12", "a22")):
        s = st(f"sf{i + 3}", 1)
        ACT(s, a[name], AF.Square)
        sqa[name] = s
    yield

    # s7: That = 2*AGA diag (into c00/c11/c22 slots)
    th = {}
    for (tag, s0, s1, s2) in (("c00", "a00", "a01", "a02"),
                              ("c11", "a01", "a11", "a12"),
                              ("c22", "a02", "a12", "a22")):
        q8 = st("tmpA", 2)
        ACT(q8, a[s0], AF.Square, scale=SQRT8)
        q = st("tmpB", 2)
        TT(q, q8, sqa[s1], AL.add)
        tt = st(tag, 1)
        TT(tt, q, sqa[s2], AL.add)
        th[tag] = tt
    yield

    # s8: That off-diag (into c01/c02/c12 slots)
    for (tag, e8t, pm, p1, p2) in (
            ("c01", e8a00, "a01", ("a01", "a11"), ("a02", "a12")),
            ("c02", e8a00, "a02", ("a01", "a12"), ("a02", "a22")),
            ("c12", e8a01, "a02", ("a11", "a12"), ("a12", "a22"))):
        m1 = st("tmpA", 2)
        TT(m1, e8t, a[pm], AL.mult)
        m2 = st("tmpB", 2)
        TT(m2, a[p1[0]], a[p1[1]], AL.mult)
        s12 = st("tmpC", 2)
        TT(s12, m1, m2, AL.add)
        m3 = st("tmpA", 2)
        TT(m3, a[p2[0]], a[p2[1]], AL.mult)
        tt = st(tag, 1)
        TT(tt, s12, m3, AL.add)
        th[tag] = tt
    yield

    # s9: S diag (into sf3..sf5 slots)
    sS = {}
    for (sname, tago, aname, thtag, g) in (
            ("s00", "sf3", "a00", "c00", g0),
            ("s11", "sf4", "a11", "c11", g12),
            ("s22", "sf5", "a22", "c22", g12)):
        k1 = st("tmpA", 2)
        TT(k1, xk20, a[aname], AL.mult)
        k2 = st("tmpB", 2)
        TT(k2, lamm, th[thtag], AL.mult)
        ks = st("tmpC", 2)
        TT(ks, k1, k2, AL.add)
        so = st(tago, 1)
        TT(so, ks, g, AL.add)
        sS[sname] = so
    yield

    # s10: S off-diag (into sf6..sf8 slots)
    for (sname, tago, aname, thtag) in (
            ("s01", "sf6", "a01", "c01"),
            ("s02", "sf7", "a02", "c02"),
            ("s12", "sf8", "a12", "c12")):
        k1 = st("tmpA", 2)
        TT(k1, xk20, a[aname], AL.mult)
        k2 = st("tmpB", 2)
        TT(k2, lamm, th[thtag], AL.mult)
        so = st(tago, 1)
        TT(so, k1, k2, AL.add)
        sS[sname] = so
    yield

    # s11..s13: P = F S (S symmetric), one output row per stage
    Smat = [[sS["s00"], sS["s01"], sS["s02"]],
            [sS["s01"], sS["s11"], sS["s12"]],
            [sS["s02"], sS["s12"], sS["s22"]]]
    for r in range(3):
        for j in range(3):
            m1 = st("tmpA", 2)
            TT(m1, f[3 * r + 0], Smat[0][j], AL.mult)
            m2 = st("tmpB", 2)
            TT(m2, f[3 * r + 1], Smat[1][j], AL.mult)
            s12 = st("tmpC", 2)
            TT(s12, m1, m2, AL.add)
            m3 = st("tmpA", 2)
            TT(m3, f[3 * r + 2], Smat[2][j], AL.mult)
            TT(pwr(3 * r + j), s12, m3, AL.add)
        yield


def _build():
    import concourse.bass as bass
    import concourse.tile as tile
    from concourse import bacc, mybir
    from contextlib import ExitStack

    f16 = mybir.dt.float16
    AL = mybir.AluOpType
    AF = mybir.ActivationFunctionType

    ROW = NT * 9 * K
    nc = bacc.Bacc("TRN2", target_bir_lowering=False, debug=False)
    fin_d = nc.dram_tensor("fin", [PART, ROW], f16, kind="ExternalInput").ap()
    pout_d = nc.dram_tensor("pout", [PART, ROW], f16, kind="ExternalOutput").ap()

    with tile.TileContext(nc) as tc:
        with ExitStack() as ctx:
            io = ctx.enter_context(tc.tile_pool(name="io", bufs=2))
            sp = ctx.enter_context(tc.tile_pool(name="sp", bufs=1))

            # Issue all input DMAs up front: the tile-t+1 load must not queue
            # behind the tile-t store's semaphore wait on the SP sequencer.
            fcs = []
            for t in range(NT):
                ft = io.tile([PART, 9 * K], f16, name="fin", tag="fin", bufs=2)
                nc.sync.dma_start(ft, fin_d[:, t * 9 * K:(t + 1) * 9 * K])
                fcs.append(ft)

            for t in range(NT):
                fc = fcs[t]
                pc = io.tile([PART, 9 * K], f16, name="pout", tag="pout",
                             bufs=1)

                slices = [("A", nc.vector, 0, KD)]
                if KG:
                    slices.append(("B", nc.gpsimd, KD, KG))
                gens = []
                for (sfx, eng, lo, w) in slices:
                    fv = [fc[:, i * K + lo: i * K + lo + w] for i in range(9)]

                    def st(tag, bufs, wide=False, _sfx=sfx, _w=w):
                        nm = f"{tag}{_sfx}"
                        dt = mybir.dt.float32 if wide else f16
                        return sp.tile([PART, _w], dt, name=nm, tag=nm, bufs=bufs)

                    def pwr(idx, _lo=lo, _w=w):
                        return pc[:, idx * K + _lo: idx * K + _lo + _w]

                    gens.append(_emit_slice_stages(nc, eng, st, fv, pwr, AL, AF))

                # Round-robin the stage emission across the engine slices.
                alive = list(gens)
                while alive:
                    nxt = []
                    for g in alive:
                        try:
                            next(g)
                            nxt.append(g)
                        except StopIteration:
                            pass
                    alive = nxt
                for r in range(3):
                    nc.sync.dma_start(
                        pout_d[:, t * 9 * K + 3 * r * K: t * 9 * K + (3 * r + 3) * K],
                        pc[:, 3 * r * K:(3 * r + 3) * K])

    nc.compile()
    return nc


def _get_nc():
    if "nc" not in _cache:
        _cache["nc"] = _build()
    return _cache["nc"]


def _make_in_maps(F):
    x = F.reshape(N, 9).astype(np.float16)
    eye9 = np.array([1, 0, 0, 0, 1, 0, 0, 0, 1], dtype=np.float16)
    pad = np.tile(eye9, (NPADPC - NPC, 1))
    in_maps = []
    for cidx in range(NCORES):
        xc = x[cidx * NPC:(cidx + 1) * NPC]
        xcp = (np.concatenate([xc, pad], axis=0)
               .reshape(PART, NT, K, 9).transpose(0, 1, 3, 2)
               .reshape(PART, NT * 9 * K))
        in_maps.append({"fin": np.ascontiguousarray(xcp)})
    return in_maps


def kernel(**inputs):
    from concourse.bass_utils import run_bass_kernel_spmd

    F = np.asarray(inputs["F"], dtype=np.float32)
    nc = _get_nc()
    in_maps = _make_in_maps(F)

    res = run_bass_kernel_spmd(nc, in_maps, list(range(NCORES)))

    out = np.empty((N, 9), dtype=np.float32)
    for cidx in range(NCORES):
        oc = (np.asarray(res.results[cidx]["pout"]).astype(np.float32)
              .reshape(PART, NT, 9, K).transpose(0, 1, 3, 2)
              .reshape(NPADPC, 9))
        out[cidx * NPC:(cidx + 1) * NPC] = oc[:NPC]
    return out.reshape(N, 3, 3)
